# revision 7
# baseline (speedup 1.0000x reference)
"""Trainium2 Bass kernel for nn_GunnarODE: neural CDE with hermite spline control.

Contract: kernel(**inputs) takes FULL unsharded inputs (ts, us, ys, W1, b1,
W2, b2, batch_size) and returns the FULL (B, L, Y) output. Internally shards
the batch across 8 NeuronCores (pure data parallel), runs a Bass/Tile kernel
per core, and reassembles.

Algorithm notes (derived from the reference):
  - x = concat([t, us]) with unit-spaced knots (ts is arange) => dt == 1.
  - Hermite backward-difference spline derivative at substep s_i = i/4 of
    interval k reduces to dXdt_i = alpha_i * slope_{k-1} + beta_i * slope_k
    with alpha_i + beta_i == 1, i.e. dXdt_i = P + beta_i * (Q - P) where
    P = slope_{k-1}, Q = slope_k; the time channel has dXdt == 1.
  - Per Euler substep: h = tanh(z@W1.T+b1); vf = tanh(h@W2.T+b2) viewed as
    (Y=16, C=9); z += 0.25 * einsum(vf, dXdt).
  - hpre = W1 @ z is THE persistent state (PSUM); z is reconstructed once per
    interval via pinv(W1) for output only.
  - fp32 matmul costs 4 cycles per moving row on the PE regardless of K/M, so
    the kernel is shaped to 3 full-batch matmuls per substep: W2a (128 rows of
    vf), W2b (16 time rows), and ONE state-update matmul with K=80 made of
    [channel-pair-summed tmp (64 rows); tanh time rows (16)]. The spline
    derivative dX is built on the Pool engine from pre-broadcast slopes, and
    the channel pair-sum runs on Vector/Pool (base-partition rules: two SBUF
    inputs must share a base partition; in/out bases may differ).
  - Batch is processed as 2 independent 256-column streams per core so the
    per-substep dependency chain of one stream hides under the other's PE work.
  - All matmuls are fp32: the ODE amplifies per-step rounding ~1e5x, so
    reduced-precision matmuls (fp32r/bf16) fail the accuracy budget.
"""
import sys
if '/opt/trn_rl_repo' not in sys.path:
    sys.path.insert(0, '/opt/trn_rl_repo')

import numpy as np

N_CORES = 8
L = 512
B_TOT = 4096
U = 8
Y = 16
H = 128
C = U + 1
NI = L - 1            # intervals
HSTEP = 0.25          # dt / SUBSTEPS with dt == 1
B_LOC = B_TOT // N_CORES  # 512
NS = 2                # independent column streams per core
BS = B_LOC // NS      # 256 columns per stream

BETA = [0.0, 0.8125, 1.25, 1.3125]

_BUILD_CACHE = {}


def _host_constants(W1, b1, W2, b2):
    """Precompute transposed/permuted constant matrices (host-side, free)."""
    # vf row r = (c-1)*16 + y  <-  W2 row y*9 + c, channels c = 1..8
    rowmap = np.array([(r % 16) * 9 + (r // 16 + 1) for r in range(128)])
    cst = {}
    cst["W1T"] = np.ascontiguousarray(W1.T)                        # (16,128)
    cst["W2aT"] = np.ascontiguousarray(W2[rowmap, :].T)            # (128,128)
    cst["W2bT"] = np.ascontiguousarray(W2[np.arange(16) * 9, :].T)  # (128,16)
    cst["b1c"] = np.ascontiguousarray(b1[:, None])                 # (128,1)
    cst["b2c"] = np.ascontiguousarray(b2[rowmap][:, None])         # (128,1)
    cst["b2t"] = np.ascontiguousarray(b2[np.arange(16) * 9][:, None])  # (16,1)
    # state-update lhsT M80 (K=80, M=128):
    # rows 0..63: pair-sum rows j=(c-1)*16+y (c=1..4 paired with c+4) -> h*W1[:, y]
    # rows 64..79: time rows y -> h*W1[:, y]
    m80 = np.zeros((80, 128), dtype=np.float32)
    for j in range(64):
        m80[j, :] = HSTEP * W1[:, j % 16]
    for y in range(16):
        m80[64 + y, :] = HSTEP * W1[:, y]
    cst["M80"] = m80
    # output reconstruction: z = pinv(W1) @ hpre  (W1 is 128x16, cond ~2)
    R = np.linalg.pinv(W1.astype(np.float64)).astype(np.float32)   # (16,128)
    cst["RT"] = np.ascontiguousarray(R.T)                          # (128,16)
    return {k: v.astype(np.float32) for k, v in cst.items()}


def _build(n_intervals=NI):
    """Build + compile the Bass module (cached per interval count)."""
    key = n_intervals
    if key in _BUILD_CACHE:
        return _BUILD_CACHE[key]

    import concourse.bass as bass
    import concourse.bacc as bacc
    import concourse.tile as tile
    from concourse import mybir

    F32 = mybir.dt.float32
    TANH = mybir.ActivationFunctionType.Tanh
    MULT = mybir.AluOpType.mult
    ADD = mybir.AluOpType.add
    SUB = mybir.AluOpType.subtract

    nc = bacc.Bacc("TRN2", target_bir_lowering=False, debug=False,
                   num_devices=N_CORES)

    d_sl = nc.dram_tensor("sl128", (n_intervals, 128, B_LOC), F32, kind="ExternalInput")
    d_ys0 = nc.dram_tensor("ys0T", (16, B_LOC), F32, kind="ExternalInput")
    d_W1T = nc.dram_tensor("W1T", (16, 128), F32, kind="ExternalInput")
    d_W2aT = nc.dram_tensor("W2aT", (128, 128), F32, kind="ExternalInput")
    d_W2bT = nc.dram_tensor("W2bT", (128, 16), F32, kind="ExternalInput")
    d_M80 = nc.dram_tensor("M80", (80, 128), F32, kind="ExternalInput")
    d_b1 = nc.dram_tensor("b1c", (128, 1), F32, kind="ExternalInput")
    d_b2c = nc.dram_tensor("b2c", (128, 1), F32, kind="ExternalInput")
    d_b2t = nc.dram_tensor("b2t", (16, 1), F32, kind="ExternalInput")
    d_RT = nc.dram_tensor("RT", (128, 16), F32, kind="ExternalInput")
    d_out = nc.dram_tensor("out", (n_intervals, NS, 16, BS), F32, kind="ExternalOutput")

    with tile.TileContext(nc) as tc:
        with (
            tc.tile_pool(name="consts", bufs=1) as consts,
            tc.tile_pool(name="qp", bufs=3) as qp,
            tc.tile_pool(name="dxp", bufs=2) as dxp,
            tc.tile_pool(name="thp", bufs=2) as thp,
            tc.tile_pool(name="vfcp", bufs=2) as vfcp,
            tc.tile_pool(name="tailp", bufs=2) as tailp,
            tc.tile_pool(name="outp", bufs=2) as outp,
            tc.tile_pool(name="psA", bufs=1, space="PSUM") as psA,
            tc.tile_pool(name="psB", bufs=1, space="PSUM") as psB,
            tc.tile_pool(name="psC", bufs=1, space="PSUM") as psC,
        ):
            W1T = consts.tile([16, 128], F32)
            W2aT = consts.tile([128, 128], F32)
            W2bT = consts.tile([128, 16], F32)
            M80 = consts.tile([80, 128], F32)
            b1c = consts.tile([128, 1], F32)
            b2c = consts.tile([128, 1], F32)
            b2t = consts.tile([16, 1], F32)
            RT = consts.tile([128, 16], F32)
            nc.sync.dma_start(W1T[:], d_W1T.ap())
            nc.sync.dma_start(W2aT[:], d_W2aT.ap())
            nc.sync.dma_start(W2bT[:], d_W2bT.ap())
            nc.sync.dma_start(M80[:], d_M80.ap())
            nc.sync.dma_start(b1c[:], d_b1.ap())
            nc.sync.dma_start(b2c[:], d_b2c.ap())
            nc.sync.dma_start(b2t[:], d_b2t.ap())
            nc.sync.dma_start(RT[:], d_RT.ap())

            z0 = consts.tile([16, B_LOC], F32)
            nc.sync.dma_start(z0[:], d_ys0.ap())

            # persistent per-stream PSUM state hpre = W1 @ z
            hpre = [psA.tile([128, BS], F32, name=f"hpre{s}") for s in range(NS)]
            for s in range(NS):
                nc.tensor.matmul(hpre[s][:], W1T[:], z0[:, s * BS:(s + 1) * BS],
                                 start=True, stop=False, skip_group_check=True)

            q_tiles = {}

            def load_q(k):
                if k < n_intervals:
                    t = qp.tile([128, B_LOC], F32, tag="q", name=f"q_{k}")
                    nc.sync.dma_start(t[:], d_sl.ap()[k])
                    q_tiles[k] = t

            load_q(0)
            load_q(1)
            for k in range(n_intervals):
                load_q(k + 2)
                Q = q_tiles[k]
                P = q_tiles.pop(k - 1) if k > 0 else Q
                if k > 0:
                    # dX_i = P + beta_i * (Q - P); dX_0 = P
                    D = dxp.tile([128, B_LOC], F32, tag="D")
                    nc.gpsimd.tensor_tensor(D[:], Q[:], P[:], SUB)
                    dXs = [P]
                    for i in (1, 2, 3):
                        dxi = dxp.tile([128, B_LOC], F32, tag=f"dx{i}")
                        nc.vector.scalar_tensor_tensor(dxi[:], D[:], BETA[i], P[:],
                                                       MULT, ADD)
                        dXs.append(dxi)
                else:
                    dXs = [Q, Q, Q, Q]

                for i in range(4):
                    dX = dXs[i]
                    ths, vfps, vtps, vfcs, upds, tlos, this_ = [], [], [], [], [], [], []
                    for s in range(NS):
                        th = thp.tile([128, BS], F32, tag=f"th{s}")
                        nc.scalar.activation(th[:], hpre[s][:], TANH, bias=b1c[:])
                        ths.append(th)
                    for s in range(NS):
                        vfp = psB.tile([128, BS], F32, tag=f"vfc{s}")
                        nc.tensor.matmul(vfp[:], W2aT[:], ths[s][:],
                                         start=True, stop=True)
                        vtp = psC.tile([16, BS], F32, tag=f"vft{s}")
                        nc.tensor.matmul(vtp[:], W2bT[:], ths[s][:],
                                         start=True, stop=True)
                        vfps.append(vfp)
                        vtps.append(vtp)
                    for s in range(NS):
                        vfc = vfcp.tile([128, BS], F32, tag=f"vfcs{s}")
                        nc.scalar.activation(vfc[:], vfps[s][:], TANH, bias=b2c[:])
                        upd = tailp.tile([80, BS], F32, tag=f"upd{s}")
                        nc.scalar.activation(upd[64:80, :], vtps[s][:], TANH,
                                             bias=b2t[:])
                        vfcs.append(vfc)
                        upds.append(upd)
                    cols = [slice(s * BS, (s + 1) * BS) for s in range(NS)]
                    for s in range(NS):
                        tlo = tailp.tile([64, BS], F32, tag=f"tlo{s}")
                        nc.vector.tensor_tensor(tlo[:], vfcs[s][0:64, :],
                                                dX[0:64, cols[s]], MULT)
                        thi = tailp.tile([64, BS], F32, tag=f"thi{s}")
                        nc.gpsimd.tensor_tensor(thi[:], vfcs[s][64:128, :],
                                                dX[64:128, cols[s]], MULT)
                        tlos.append(tlo)
                        this_.append(thi)
                    for s in range(NS):
                        nc.gpsimd.tensor_tensor(upds[s][0:64, :], tlos[s][:],
                                                this_[s][:], ADD)
                    for s in range(NS):
                        nc.tensor.matmul(hpre[s][:], M80[:], upds[s][0:80, :],
                                         start=False, stop=False,
                                         skip_group_check=True)

                # per-interval output: z_{k+1} = pinv(W1) @ hpre
                for s in range(NS):
                    hps = outp.tile([128, BS], F32, tag=f"hps{s}")
                    nc.vector.tensor_copy(hps[:], hpre[s][:])
                    ztp = psC.tile([16, BS], F32, tag=f"vft{s}")
                    nc.tensor.matmul(ztp[:], RT[:], hps[:], start=True, stop=True)
                    zout = outp.tile([16, BS], F32, tag=f"zo{s}")
                    nc.vector.tensor_copy(zout[:], ztp[:])
                    nc.sync.dma_start(d_out.ap()[k][s], zout[:])

    nc.compile()
    _BUILD_CACHE[key] = nc
    return nc


def _prep_core_inputs(slopes, ys, cst, core, n_intervals):
    b0 = core * B_LOC
    # slopes: (L-1, B_TOT, 8) -> per-core (n_intervals, 8, B_LOC) -> repeat 16x
    sl = np.ascontiguousarray(
        slopes[:n_intervals, b0:b0 + B_LOC, :].transpose(0, 2, 1))
    sl128 = np.repeat(sl, 16, axis=1)                # (NI, 128, B_LOC)
    ys0T = np.ascontiguousarray(ys[0, b0:b0 + B_LOC, :].T).astype(np.float32)
    m = {"sl128": np.ascontiguousarray(sl128), "ys0T": ys0T}
    m.update(cst)
    return m


def kernel(ts, us, ys, W1, b1, W2, b2, batch_size=None, n_intervals=NI):
    from concourse.bass_utils import run_bass_kernel_spmd

    us = np.asarray(us, dtype=np.float32)
    ys = np.asarray(ys, dtype=np.float32)
    cst = _host_constants(np.asarray(W1, np.float32), np.asarray(b1, np.float32),
                          np.asarray(W2, np.float32), np.asarray(b2, np.float32))
    slopes = us[1:] - us[:-1]                        # (L-1, B, 8)
    nc = _build(n_intervals)
    in_maps = [_prep_core_inputs(slopes, ys, cst, c, n_intervals)
               for c in range(N_CORES)]
    res = run_bass_kernel_spmd(nc, in_maps, core_ids=list(range(N_CORES)))
    out = np.empty((B_TOT, n_intervals + 1, Y), dtype=np.float32)
    out[:, 0, :] = ys[0]
    for c in range(N_CORES):
        b0 = c * B_LOC
        r = res.results[c]["out"]                    # (NI, NS, 16, BS)
        out[b0:b0 + B_LOC, 1:, :] = r.transpose(1, 3, 0, 2).reshape(
            B_LOC, n_intervals, Y)
    kernel._last_results = res
    return out
